# revision 7
# baseline (speedup 1.0000x reference)
"""Trainium2 distributed kernel for nn_ActorGNNMLP (3-layer hetero GraphConv + MLP).

Approach
--------
Each DGL GraphConv is linear:  gconv(x) = S @ (x @ W) + b  with
S = D_in^-1/2 A D_out^-1/2 a dense normalized adjacency built host-side from
the edge indices only (pure index/layout preprocessing; all feature compute
runs on device).  Work is sharded over 8 NeuronCores by destination rows
(region/driver padded 5000 -> 5120 = 8*640).  Hidden states are kept in a
transposed [feat, node] layout on-chip so no on-device transposes are needed;
full hidden states are exchanged between layers with AllGather collectives.
"""

import sys

sys.path.insert(0, "/opt/trn_rl_repo")

import numpy as np
import ml_dtypes

from concourse import bass, bacc, mybir, tile
from concourse.bass_utils import run_bass_kernel_spmd

BF16 = ml_dtypes.bfloat16
F32 = mybir.dt.float32
BF = mybir.dt.bfloat16

NC = 8
NR, ND, NO, NPOI = 5000, 5000, 50000, 20000
PN = 5120          # padded region/driver count
SH = PN // NC      # 640 dst rows per core
KO = 50048         # padded order count   (391 * 128)
KP = 20096         # padded poi count     (157 * 128)
FD = 5025
FDP = 5120         # padded driver feature dim
HID, EMB, MLPH, ACT = 128, 64, 256, 26
NT = PN // 128     # 40 src tiles (region/driver)
NTO = KO // 128    # 391
NTP = KP // 128    # 157
NB = SH // 128     # 5 128-blocks per shard

_CACHE = {}


# --------------------------------------------------------------------------
# host-side graph preprocessing (indices only)
# --------------------------------------------------------------------------

def _build_ST(src, dst, n_src, n_dst, n_src_pad):
    """S.T = (D_in^-1/2 A D_out^-1/2).T as [n_src_pad, PN] float32."""
    AT = np.zeros((n_src_pad, PN), np.float32)
    np.add.at(AT, (src, dst), 1.0)
    dout = np.maximum(np.bincount(src, minlength=n_src), 1).astype(np.float32) ** -0.5
    din = np.maximum(np.bincount(dst, minlength=n_dst), 1).astype(np.float32) ** -0.5
    AT[:n_src] *= dout[:, None]
    AT[:, :n_dst] *= din[None, :]
    return AT


def _shard_cols(M16):
    return [np.ascontiguousarray(M16[:, c * SH:(c + 1) * SH]) for c in range(NC)]


def _img(x, n_pad, f):
    """[n, f] -> partition-major SBUF image [128, (n_pad/128)*f] float32."""
    xp = np.zeros((n_pad, f), np.float32)
    xp[:x.shape[0]] = x
    nt = n_pad // 128
    return np.ascontiguousarray(
        xp.reshape(nt, 128, f).transpose(1, 0, 2).reshape(128, nt * f))


def _prep_inputs(x_region, x_driver, x_order, x_poi,
                 r2r_src, r2r_dst, d2r_src, d2r_dst, d2d_src, d2d_dst,
                 r2d_src, r2d_dst, o2r_src, o2r_dst, p2r_src, p2r_dst, params):
    p = {k: np.asarray(v, np.float32) for k, v in params.items()}

    sh = {}
    for name, (s, d, ns, nd_, npd) in {
        "srrT": (r2r_src, r2r_dst, NR, NR, PN),
        "sdrT": (d2r_src, d2r_dst, ND, NR, PN),
        "sddT": (d2d_src, d2d_dst, ND, ND, PN),
        "srdT": (r2d_src, r2d_dst, NR, ND, PN),
        "sorT": (o2r_src, o2r_dst, NO, NR, KO),
        "sprT": (p2r_src, p2r_dst, NPOI, NR, KP),
    }.items():
        ST = _build_ST(np.asarray(s), np.asarray(d), ns, nd_, npd).astype(BF16)
        sh[name] = _shard_cols(ST)
        del ST

    # x_driver.T padded [FDP, PN], sharded by driver (columns)
    xdT = np.zeros((FDP, PN), np.float32)
    xdT[:FD, :ND] = np.asarray(x_driver, np.float32).T
    sh["xdT"] = _shard_cols(xdT)
    del xdT

    xrT = np.zeros((4, PN), np.float32)
    xrT[:, :NR] = np.asarray(x_region, np.float32).T

    rep = {
        "xrT": xrT,
        "xo_img": _img(np.asarray(x_order, np.float32), KO, 6),
        "xp_img": _img(np.asarray(x_poi, np.float32), KP, 4),
        "w1d": np.ascontiguousarray(
            np.concatenate([
                np.pad(p["W1_dr"], ((0, FDP - FD), (0, 0))),
                np.pad(p["W1_dd"], ((0, FDP - FD), (0, 0)))], axis=1)),
        "w1r": np.ascontiguousarray(np.concatenate([p["W1_rr"], p["W1_rd"]], axis=1)),
        "w1o": np.ascontiguousarray(p["W1_or"]),
        "w1p": np.ascontiguousarray(p["W1_pr"]),
        "w2r": np.ascontiguousarray(np.concatenate([p["W2_rr"], p["W2_rd"]], axis=1)),
        "w2d": np.ascontiguousarray(np.concatenate([p["W2_dd"], p["W2_dr"]], axis=1)),
        "w3r": np.ascontiguousarray(p["W3_rd"]),
        "w3d": np.ascontiguousarray(p["W3_dd"]),
        "wf1": np.ascontiguousarray(p["Wf1"]),
        "wf2_img": np.ascontiguousarray(
            p["Wf2"].reshape(2, 128, MLPH).transpose(1, 0, 2).reshape(128, 2 * MLPH)),
        "wf3_img": np.ascontiguousarray(
            p["Wf3"].reshape(2, 128, ACT).transpose(1, 0, 2).reshape(128, 2 * ACT)),
        "b128": np.ascontiguousarray(np.stack([
            p["b1_rr"], p["b1_dr"], p["b1_or"], p["b1_pr"],
            p["b1_dd"], p["b1_rd"],
            p["b2_rr"], p["b2_dr"], p["b2_dd"], p["b2_rd"]])),
        "b64": np.ascontiguousarray(np.stack([p["b3_dd"], p["b3_rd"]])),
        "bf12": np.ascontiguousarray(np.stack([p["bf1"], p["bf2"]])),
        "bf3v": np.ascontiguousarray(p["bf3"][None, :]),
    }

    in_maps = []
    for c in range(NC):
        m = {k: v[c] for k, v in sh.items()}
        m.update(rep)
        in_maps.append(m)
    return in_maps


# --------------------------------------------------------------------------
# device kernel
# --------------------------------------------------------------------------

def _declare(nc):
    d = {}
    specs = {
        "srrT": ([PN, SH], BF), "sdrT": ([PN, SH], BF),
        "sddT": ([PN, SH], BF), "srdT": ([PN, SH], BF),
        "sorT": ([KO, SH], BF), "sprT": ([KP, SH], BF),
        "xdT": ([FDP, SH], F32), "xrT": ([4, PN], F32),
        "xo_img": ([128, NTO * 6], F32), "xp_img": ([128, NTP * 4], F32),
        "w1d": ([FDP, 256], F32), "w1r": ([4, 256], F32),
        "w1o": ([6, 128], F32), "w1p": ([4, 128], F32),
        "w2r": ([128, 256], F32), "w2d": ([128, 256], F32),
        "w3r": ([128, 64], F32), "w3d": ([128, 64], F32),
        "wf1": ([64, 256], F32), "wf2_img": ([128, 512], F32),
        "wf3_img": ([128, 2 * ACT], F32),
        "b128": ([10, 128], F32), "b64": ([2, 64], F32),
        "bf12": ([2, 256], F32), "bf3v": ([1, ACT], F32),
    }
    for k, (shape, dt) in specs.items():
        d[k] = nc.dram_tensor(k, shape, dt, kind="ExternalInput")
    d["out"] = nc.dram_tensor("out", [ACT + EMB, SH], F32, kind="ExternalOutput")
    return d


def _build(nc):
    d = _declare(nc)
    RG = [list(range(NC))]

    with tile.TileContext(nc) as tc:
        with (
            tc.tile_pool(name="const", bufs=1) as cp,
            tc.tile_pool(name="dram", bufs=1, space="DRAM") as dp,
        ):
            # ---- constants -------------------------------------------------
            ones = cp.tile([26, 1], F32, name="ones")
            nc.vector.memset(ones[:], 1.0)

            bvecs = {}
            with tc.tile_pool(name="cpsum", bufs=2, space="PSUM") as cps:
                def bias_vec(name, dram_ap, nrows, rows):
                    # load the group at partition 0, then [nrows,C] -> [C,1]
                    # via a ones-matmul (sums the group while transposing)
                    src = cp.tile([nrows, rows], F32, name=name + "_src")
                    nc.sync.dma_start(src[:], dram_ap)
                    ps = cps.tile([rows, 1], F32, name="bps", tag="bps")
                    nc.tensor.matmul(ps[:], src[:], ones[:nrows, :],
                                     start=True, stop=True)
                    sb = cp.tile([rows, 1], F32, name=name)
                    nc.scalar.copy(sb[:], ps[:])
                    bvecs[name] = sb
                    return sb

                bias_vec("b_hr", d["b128"][0:4, :], 4, 128)
                bias_vec("b_hd", d["b128"][4:6, :], 2, 128)
                bias_vec("b_nr", d["b128"][6:8, :], 2, 128)
                bias_vec("b_nd", d["b128"][8:10, :], 2, 128)
                bias_vec("b_g", d["b64"][0:2, :], 2, 64)
                bias_vec("b_f1a", d["bf12"][0:1, 0:128], 1, 128)
                bias_vec("b_f1b", d["bf12"][0:1, 128:256], 1, 128)
                bias_vec("b_f2a", d["bf12"][1:2, 0:128], 1, 128)
                bias_vec("b_f2b", d["bf12"][1:2, 128:256], 1, 128)
                bias_vec("b_f3", d["bf3v"][0:1, :], 1, ACT)

            def load_bf(name, dram, shape):
                f = cp.tile(shape, F32, name=name + "_f")
                nc.sync.dma_start(f[:], dram[:])
                b = cp.tile(shape, BF, name=name)
                nc.vector.tensor_copy(b[:], f[:])
                return b

            w1r_bf = load_bf("w1r_bf", d["w1r"], [4, 256])
            w1o_bf = load_bf("w1o_bf", d["w1o"], [6, 128])
            w1p_bf = load_bf("w1p_bf", d["w1p"], [4, 128])
            w2r_bf = load_bf("w2r_bf", d["w2r"], [128, 256])
            w2d_bf = load_bf("w2d_bf", d["w2d"], [128, 256])
            w3r_bf = load_bf("w3r_bf", d["w3r"], [128, 64])
            w3d_bf = load_bf("w3d_bf", d["w3d"], [128, 64])
            wf1_bf = load_bf("wf1_bf", d["wf1"], [64, 256])
            wf2_bf = load_bf("wf2_bf", d["wf2_img"], [128, 512])
            wf3_bf = load_bf("wf3_bf", d["wf3_img"], [128, 2 * ACT])

            xr_bf = load_bf("xr_bf", d["xrT"], [4, PN])
            xo_bf = load_bf("xo_bf", d["xo_img"], [128, NTO * 6])
            xp_bf = load_bf("xp_bf", d["xp_img"], [128, NTP * 4])

            # ---- collective buffers ---------------------------------------
            cc_td_in = dp.tile([SH, 256], BF, name="cc_td_in")
            cc_td_out = dp.tile([PN, 256], BF, name="cc_td_out",
                                addr_space="Shared")
            cc = {}
            for h in ("hr", "hd", "hr2", "hd2"):
                cc[h + "_in"] = dp.tile([128, SH], BF, name=f"cc_{h}_in")
                cc[h + "_out"] = dp.tile([128 * NC, SH], BF, name=f"cc_{h}_out",
                                         addr_space="Shared")

            # ---- phase A: driver transform + AG ---------------------------
            with (
                tc.tile_pool(name="xd", bufs=3) as xdp,
                tc.tile_pool(name="tdps", bufs=1, space="PSUM") as tdpsp,
                tc.tile_pool(name="tdsb", bufs=1) as tdsbp,
            ):
                td_ps = [tdpsp.tile([128, 256], F32, name=f"td_ps{m}")
                         for m in range(NB)]
                for k in range(NT):
                    xdf = xdp.tile([128, SH], F32, name="xdf")
                    nc.sync.dma_start(xdf[:], d["xdT"][k * 128:(k + 1) * 128, :])
                    xdb = xdp.tile([128, SH], BF, name="xdb")
                    nc.vector.tensor_copy(xdb[:], xdf[:])
                    w1f = xdp.tile([128, 256], F32, name="w1f")
                    nc.sync.dma_start(w1f[:], d["w1d"][k * 128:(k + 1) * 128, :])
                    w1b = xdp.tile([128, 256], BF, name="w1b")
                    nc.vector.tensor_copy(w1b[:], w1f[:])
                    for m in range(NB):
                        nc.tensor.matmul(td_ps[m][:],
                                         xdb[:, m * 128:(m + 1) * 128], w1b[:],
                                         start=(k == 0), stop=(k == NT - 1))
                for m in range(NB):
                    tds = tdsbp.tile([128, 256], BF, name=f"tds{m}")
                    nc.vector.tensor_copy(tds[:], td_ps[m][:])
                    nc.sync.dma_start(cc_td_in[m * 128:(m + 1) * 128, :], tds[:])
                nc.gpsimd.collective_compute(
                    "AllGather", mybir.AluOpType.bypass, replica_groups=RG,
                    ins=[cc_td_in[:].opt()], outs=[cc_td_out[:].opt()])

            # ---- phase B: layer-1 aggregation -----------------------------
            def agg_pair(ps_a, ps_b, lhsT, rhs, first):
                nc.tensor.matmul(ps_a[:], lhsT, rhs[:, 0:512],
                                 start=first, stop=False)
                nc.tensor.matmul(ps_b[:], lhsT, rhs[:, 512:SH],
                                 start=first, stop=False)

            with (
                tc.tile_pool(name="trp", bufs=1) as trp,
                tc.tile_pool(name="accps", bufs=1, space="PSUM") as accp,
                tc.tile_pool(name="sstream", bufs=6) as ssp,
                tc.tile_pool(name="hsb", bufs=1) as hsbp,
            ):
                # region transform (local, replicated — tiny)
                t_r = []
                with tc.tile_pool(name="trps", bufs=2, space="PSUM") as trps:
                    for s in range(NT):
                        ps = trps.tile([128, 256], F32, name="trps_t")
                        nc.tensor.matmul(ps[:], xr_bf[0:4, s * 128:(s + 1) * 128],
                                         w1r_bf[:], start=True, stop=True)
                        t = trp.tile([128, 256], BF, name=f"t_r{s}")
                        nc.vector.tensor_copy(t[:], ps[:])
                        t_r.append(t)
                hr_a = accp.tile([128, 512], F32, name="hr_a")
                hr_b = accp.tile([128, SH - 512], F32, name="hr_b")
                hd_a = accp.tile([128, 512], F32, name="hd_a")
                hd_b = accp.tile([128, SH - 512], F32, name="hd_b")

                for s in range(NT):
                    srr = ssp.tile([128, SH], BF, name="srr")
                    nc.sync.dma_start(srr[:], d["srrT"][s * 128:(s + 1) * 128, :])
                    agg_pair(hr_a, hr_b, t_r[s][:, 0:128], srr, s == 0)
                    srd = ssp.tile([128, SH], BF, name="srd")
                    nc.sync.dma_start(srd[:], d["srdT"][s * 128:(s + 1) * 128, :])
                    agg_pair(hd_a, hd_b, t_r[s][:, 128:256], srd, s == 0)
                    tdf = ssp.tile([128, 256], BF, name="tdf")
                    nc.sync.dma_start(tdf[:], cc_td_out[s * 128:(s + 1) * 128, :])
                    sdr = ssp.tile([128, SH], BF, name="sdr")
                    nc.sync.dma_start(sdr[:], d["sdrT"][s * 128:(s + 1) * 128, :])
                    agg_pair(hr_a, hr_b, tdf[:, 0:128], sdr, False)
                    sdd = ssp.tile([128, SH], BF, name="sdd")
                    nc.sync.dma_start(sdd[:], d["sddT"][s * 128:(s + 1) * 128, :])
                    agg_pair(hd_a, hd_b, tdf[:, 128:256], sdd, False)

                # o2r:  z = (S_or @ x_order).T  then  W1_or.T @ z -> hrT
                with tc.tile_pool(name="zps", bufs=1, space="PSUM") as zpsp:
                    zo_a = zpsp.tile([6, 512], F32, name="zo_a")
                    zo_b = zpsp.tile([6, SH - 512], F32, name="zo_b")
                    for k in range(NTO):
                        so = ssp.tile([128, SH], BF, name="srr")
                        nc.sync.dma_start(so[:], d["sorT"][k * 128:(k + 1) * 128, :])
                        nc.tensor.matmul(zo_a[:], xo_bf[:, k * 6:(k + 1) * 6],
                                         so[:, 0:512], start=(k == 0),
                                         stop=(k == NTO - 1))
                        nc.tensor.matmul(zo_b[:], xo_bf[:, k * 6:(k + 1) * 6],
                                         so[:, 512:SH], start=(k == 0),
                                         stop=(k == NTO - 1))
                    zo = hsbp.tile([6, SH], BF, name="zo")
                    nc.vector.tensor_copy(zo[:, 0:512], zo_a[:])
                    nc.vector.tensor_copy(zo[:, 512:SH], zo_b[:])
                    agg_pair(hr_a, hr_b, w1o_bf[:], zo, False)

                    zp_a = zpsp.tile([4, 512], F32, name="zp_a", tag="zo_a")
                    zp_b = zpsp.tile([4, SH - 512], F32, name="zp_b", tag="zo_b")
                    for k in range(NTP):
                        sp_ = ssp.tile([128, SH], BF, name="srr")
                        nc.sync.dma_start(sp_[:], d["sprT"][k * 128:(k + 1) * 128, :])
                        nc.tensor.matmul(zp_a[:], xp_bf[:, k * 4:(k + 1) * 4],
                                         sp_[:, 0:512], start=(k == 0),
                                         stop=(k == NTP - 1))
                        nc.tensor.matmul(zp_b[:], xp_bf[:, k * 4:(k + 1) * 4],
                                         sp_[:, 512:SH], start=(k == 0),
                                         stop=(k == NTP - 1))
                    zp = hsbp.tile([4, SH], BF, name="zp")
                    nc.vector.tensor_copy(zp[:, 0:512], zp_a[:])
                    nc.vector.tensor_copy(zp[:, 512:SH], zp_b[:])
                    agg_pair(hr_a, hr_b, w1p_bf[:], zp, False)

                hr_sb = hsbp.tile([128, SH], BF, name="hr_sb")
                nc.scalar.activation(hr_sb[:, 0:512], hr_a[:],
                                     mybir.ActivationFunctionType.Relu,
                                     bias=bvecs["b_hr"][:])
                nc.scalar.activation(hr_sb[:, 512:SH], hr_b[:],
                                     mybir.ActivationFunctionType.Relu,
                                     bias=bvecs["b_hr"][:])
                hd_sb = hsbp.tile([128, SH], BF, name="hd_sb")
                nc.scalar.activation(hd_sb[:, 0:512], hd_a[:],
                                     mybir.ActivationFunctionType.Relu,
                                     bias=bvecs["b_hd"][:])
                nc.scalar.activation(hd_sb[:, 512:SH], hd_b[:],
                                     mybir.ActivationFunctionType.Relu,
                                     bias=bvecs["b_hd"][:])
                nc.sync.dma_start(cc["hr_in"][:], hr_sb[:])
                nc.sync.dma_start(cc["hd_in"][:], hd_sb[:])
                nc.gpsimd.collective_compute(
                    "AllGather", mybir.AluOpType.bypass, replica_groups=RG,
                    ins=[cc["hr_in"][:].opt()], outs=[cc["hr_out"][:].opt()])
                nc.gpsimd.collective_compute(
                    "AllGather", mybir.AluOpType.bypass, replica_groups=RG,
                    ins=[cc["hd_in"][:].opt()], outs=[cc["hd_out"][:].opt()])

            # ---- phase C: layer 2 -----------------------------------------
            with (
                tc.tile_pool(name="hblk", bufs=1) as hbp,
                tc.tile_pool(name="t2", bufs=1) as t2p,
                tc.tile_pool(name="t2ps", bufs=4, space="PSUM") as t2psp,
                tc.tile_pool(name="sstream2", bufs=6) as ssp2,
                tc.tile_pool(name="acc2", bufs=1, space="PSUM") as accp2,
                tc.tile_pool(name="hsb2", bufs=1) as hsbp2,
            ):
                hrB, hdB = [], []
                for c in range(NC):
                    hb = hbp.tile([128, SH], BF, name=f"hrB{c}")
                    nc.sync.dma_start(hb[:], cc["hr_out"][c * 128:(c + 1) * 128, :])
                    hrB.append(hb)
                    db = hbp.tile([128, SH], BF, name=f"hdB{c}")
                    nc.sync.dma_start(db[:], cc["hd_out"][c * 128:(c + 1) * 128, :])
                    hdB.append(db)

                t2r, t2d = [], []
                for s in range(NT):
                    c, sub = divmod(s, NB)
                    ps = t2psp.tile([128, 256], F32, name="t2ps_t")
                    nc.tensor.matmul(ps[:], hrB[c][:, sub * 128:(sub + 1) * 128],
                                     w2r_bf[:], start=True, stop=True)
                    t = t2p.tile([128, 256], BF, name=f"t2r{s}")
                    nc.vector.tensor_copy(t[:], ps[:])
                    t2r.append(t)
                    ps2 = t2psp.tile([128, 256], F32, name="t2ps_t")
                    nc.tensor.matmul(ps2[:], hdB[c][:, sub * 128:(sub + 1) * 128],
                                     w2d_bf[:], start=True, stop=True)
                    t2 = t2p.tile([128, 256], BF, name=f"t2d{s}")
                    nc.vector.tensor_copy(t2[:], ps2[:])
                    t2d.append(t2)

                nr_a = accp2.tile([128, 512], F32, name="nr_a")
                nr_b = accp2.tile([128, SH - 512], F32, name="nr_b")
                nd_a = accp2.tile([128, 512], F32, name="nd_a")
                nd_b = accp2.tile([128, SH - 512], F32, name="nd_b")
                for s in range(NT):
                    srr = ssp2.tile([128, SH], BF, name="s2")
                    nc.sync.dma_start(srr[:], d["srrT"][s * 128:(s + 1) * 128, :])
                    agg_pair(nr_a, nr_b, t2r[s][:, 0:128], srr, s == 0)
                    srd = ssp2.tile([128, SH], BF, name="s2")
                    nc.sync.dma_start(srd[:], d["srdT"][s * 128:(s + 1) * 128, :])
                    agg_pair(nd_a, nd_b, t2r[s][:, 128:256], srd, s == 0)
                    sdr = ssp2.tile([128, SH], BF, name="s2")
                    nc.sync.dma_start(sdr[:], d["sdrT"][s * 128:(s + 1) * 128, :])
                    agg_pair(nr_a, nr_b, t2d[s][:, 128:256], sdr, False)
                    sdd = ssp2.tile([128, SH], BF, name="s2")
                    nc.sync.dma_start(sdd[:], d["sddT"][s * 128:(s + 1) * 128, :])
                    agg_pair(nd_a, nd_b, t2d[s][:, 0:128], sdd, False)

                hr2_sb = hsbp2.tile([128, SH], BF, name="hr2_sb")
                nc.scalar.activation(hr2_sb[:, 0:512], nr_a[:],
                                     mybir.ActivationFunctionType.Relu,
                                     bias=bvecs["b_nr"][:])
                nc.scalar.activation(hr2_sb[:, 512:SH], nr_b[:],
                                     mybir.ActivationFunctionType.Relu,
                                     bias=bvecs["b_nr"][:])
                hd2_sb = hsbp2.tile([128, SH], BF, name="hd2_sb")
                nc.scalar.activation(hd2_sb[:, 0:512], nd_a[:],
                                     mybir.ActivationFunctionType.Relu,
                                     bias=bvecs["b_nd"][:])
                nc.scalar.activation(hd2_sb[:, 512:SH], nd_b[:],
                                     mybir.ActivationFunctionType.Relu,
                                     bias=bvecs["b_nd"][:])
                nc.sync.dma_start(cc["hr2_in"][:], hr2_sb[:])
                nc.sync.dma_start(cc["hd2_in"][:], hd2_sb[:])
                nc.gpsimd.collective_compute(
                    "AllGather", mybir.AluOpType.bypass, replica_groups=RG,
                    ins=[cc["hr2_in"][:].opt()], outs=[cc["hr2_out"][:].opt()])
                nc.gpsimd.collective_compute(
                    "AllGather", mybir.AluOpType.bypass, replica_groups=RG,
                    ins=[cc["hd2_in"][:].opt()], outs=[cc["hd2_out"][:].opt()])

            # ---- phase D: layer 3 (driver dst only) + MLP -----------------
            with (
                tc.tile_pool(name="hblk3", bufs=1) as hbp3,
                tc.tile_pool(name="t3", bufs=1) as t3p,
                tc.tile_pool(name="t3ps", bufs=2, space="PSUM") as t3psp,
                tc.tile_pool(name="sstream3", bufs=6) as ssp3,
                tc.tile_pool(name="acc3", bufs=1, space="PSUM") as accp3,
                tc.tile_pool(name="mlp", bufs=1) as mlpp,
                tc.tile_pool(name="mlpps", bufs=1, space="PSUM") as mlppsp,
            ):
                hr2B, hd2B = [], []
                for c in range(NC):
                    hb = hbp3.tile([128, SH], BF, name=f"hr2B{c}")
                    nc.sync.dma_start(hb[:], cc["hr2_out"][c * 128:(c + 1) * 128, :])
                    hr2B.append(hb)
                    db = hbp3.tile([128, SH], BF, name=f"hd2B{c}")
                    nc.sync.dma_start(db[:], cc["hd2_out"][c * 128:(c + 1) * 128, :])
                    hd2B.append(db)

                t3r, t3d = [], []
                for s in range(NT):
                    c, sub = divmod(s, NB)
                    ps = t3psp.tile([128, 64], F32, name="t3ps_t")
                    nc.tensor.matmul(ps[:], hr2B[c][:, sub * 128:(sub + 1) * 128],
                                     w3r_bf[:], start=True, stop=True)
                    t = t3p.tile([128, 64], BF, name=f"t3r{s}")
                    nc.vector.tensor_copy(t[:], ps[:])
                    t3r.append(t)
                    ps2 = t3psp.tile([128, 64], F32, name="t3ps_t")
                    nc.tensor.matmul(ps2[:], hd2B[c][:, sub * 128:(sub + 1) * 128],
                                     w3d_bf[:], start=True, stop=True)
                    t2 = t3p.tile([128, 64], BF, name=f"t3d{s}")
                    nc.vector.tensor_copy(t2[:], ps2[:])
                    t3d.append(t2)

                g_a = accp3.tile([64, 512], F32, name="g_a")
                g_b = accp3.tile([64, SH - 512], F32, name="g_b")
                for s in range(NT):
                    sdd = ssp3.tile([128, SH], BF, name="s3")
                    nc.sync.dma_start(sdd[:], d["sddT"][s * 128:(s + 1) * 128, :])
                    agg_pair(g_a, g_b, t3d[s][:], sdd, s == 0)
                    srd = ssp3.tile([128, SH], BF, name="s3")
                    nc.sync.dma_start(srd[:], d["srdT"][s * 128:(s + 1) * 128, :])
                    agg_pair(g_a, g_b, t3r[s][:], srd, False)

                g_bf = mlpp.tile([64, SH], BF, name="g_bf")
                g_f32 = mlpp.tile([64, SH], F32, name="g_f32")
                for (lo, hi, ps) in ((0, 512, g_a), (512, SH, g_b)):
                    nc.vector.tensor_scalar_add(g_bf[:, lo:hi], ps[:],
                                                bvecs["b_g"][:])
                    nc.scalar.activation(g_f32[:, lo:hi], ps[:],
                                         mybir.ActivationFunctionType.Copy,
                                         bias=0.0, scale=1.0)
                    nc.vector.tensor_scalar_add(g_f32[:, lo:hi], ps[:],
                                                bvecs["b_g"][:])
                nc.sync.dma_start(d["out"][ACT:ACT + EMB, :], g_f32[:])

                # MLP head (transposed layout: [hidden, node])
                x1 = []
                for h in range(2):
                    ps_a = mlppsp.tile([128, 512], F32, name="m1a", tag="mlp_a")
                    ps_b = mlppsp.tile([128, SH - 512], F32, name="m1b", tag="mlp_b")
                    nc.tensor.matmul(ps_a[:], wf1_bf[0:64, h * 128:(h + 1) * 128],
                                     g_bf[:, 0:512], start=True, stop=True)
                    nc.tensor.matmul(ps_b[:], wf1_bf[0:64, h * 128:(h + 1) * 128],
                                     g_bf[:, 512:SH], start=True, stop=True)
                    x = mlpp.tile([128, SH], BF, name=f"x1_{h}")
                    nc.scalar.activation(x[:, 0:512], ps_a[:],
                                         mybir.ActivationFunctionType.Relu,
                                         bias=bvecs["b_f1a" if h == 0 else "b_f1b"][:])
                    nc.scalar.activation(x[:, 512:SH], ps_b[:],
                                         mybir.ActivationFunctionType.Relu,
                                         bias=bvecs["b_f1a" if h == 0 else "b_f1b"][:])
                    x1.append(x)

                x2 = []
                for h in range(2):
                    ps_a = mlppsp.tile([128, 512], F32, name="m2a", tag="mlp_a")
                    ps_b = mlppsp.tile([128, SH - 512], F32, name="m2b", tag="mlp_b")
                    for k in range(2):
                        lhsT = wf2_bf[:, 256 * k + 128 * h: 256 * k + 128 * h + 128]
                        nc.tensor.matmul(ps_a[:], lhsT, x1[k][:, 0:512],
                                         start=(k == 0), stop=(k == 1))
                        nc.tensor.matmul(ps_b[:], lhsT, x1[k][:, 512:SH],
                                         start=(k == 0), stop=(k == 1))
                    x = mlpp.tile([128, SH], BF, name=f"x2_{h}")
                    nc.scalar.activation(x[:, 0:512], ps_a[:],
                                         mybir.ActivationFunctionType.Relu,
                                         bias=bvecs["b_f2a" if h == 0 else "b_f2b"][:])
                    nc.scalar.activation(x[:, 512:SH], ps_b[:],
                                         mybir.ActivationFunctionType.Relu,
                                         bias=bvecs["b_f2a" if h == 0 else "b_f2b"][:])
                    x2.append(x)

                lg_a = mlppsp.tile([ACT, 512], F32, name="lga", tag="mlp_a")
                lg_b = mlppsp.tile([ACT, SH - 512], F32, name="lgb", tag="mlp_b")
                for k in range(2):
                    lhsT = wf3_bf[:, ACT * k:ACT * (k + 1)]
                    nc.tensor.matmul(lg_a[:], lhsT, x2[k][:, 0:512],
                                     start=(k == 0), stop=(k == 1))
                    nc.tensor.matmul(lg_b[:], lhsT, x2[k][:, 512:SH],
                                     start=(k == 0), stop=(k == 1))

                e_sb = mlpp.tile([ACT, SH], F32, name="e_sb")
                nc.scalar.activation(e_sb[:, 0:512], lg_a[:],
                                     mybir.ActivationFunctionType.Exp,
                                     bias=bvecs["b_f3"][:])
                nc.scalar.activation(e_sb[:, 512:SH], lg_b[:],
                                     mybir.ActivationFunctionType.Exp,
                                     bias=bvecs["b_f3"][:])
                s_a = mlppsp.tile([1, 512], F32, name="sa", tag="mlp_a")
                s_b = mlppsp.tile([1, SH - 512], F32, name="sb", tag="mlp_b")
                nc.tensor.matmul(s_a[:], ones[:ACT, :], e_sb[:, 0:512],
                                 start=True, stop=True)
                nc.tensor.matmul(s_b[:], ones[:ACT, :], e_sb[:, 512:SH],
                                 start=True, stop=True)
                r_sb = mlpp.tile([1, SH], F32, name="r_sb")
                nc.vector.reciprocal(r_sb[:, 0:512], s_a[:])
                nc.vector.reciprocal(r_sb[:, 512:SH], s_b[:])
                r_bc = mlpp.tile([ACT, SH], F32, name="r_bc")
                nc.gpsimd.partition_broadcast(r_bc[:], r_sb[:])
                prob = mlpp.tile([ACT, SH], F32, name="prob")
                nc.vector.tensor_mul(prob[:], e_sb[:], r_bc[:])
                nc.sync.dma_start(d["out"][0:ACT, :], prob[:])

    nc.compile()
    return nc


def _get_nc():
    if "nc" not in _CACHE:
        nc = bacc.Bacc("TRN2", target_bir_lowering=False, debug=False,
                       num_devices=NC)
        _CACHE["nc"] = _build(nc)
    return _CACHE["nc"]


def _install_profile_hook():
    """Provide antenv.axon_hooks (missing in this image) so that
    run_bass_kernel_spmd(trace=True) can capture NTFF profiles via the
    axon PJRT .so; also stub out the artifact upload (no object store)."""
    import types, ctypes, contextlib

    import concourse.bass_utils as bu
    bu.upload_artifacts = lambda tmpdir: str(tmpdir)

    import antenv
    if "antenv.axon_hooks" in sys.modules:
        return
    mod = types.ModuleType("antenv.axon_hooks")
    hook_box = [None]
    mod.set_axon_ntff_profile_hook = lambda h: hook_box.__setitem__(0, h)
    mod.get_axon_ntff_profile_hook = lambda: hook_box[0]
    sys.modules["antenv.axon_hooks"] = mod
    antenv.axon_hooks = mod

    so_path = "/opt/axon/libaxon_pjrt.so"
    lib = ctypes.CDLL(so_path)
    if not hasattr(lib, "axon_start_nrt_profile"):
        return
    lib.axon_start_nrt_profile.argtypes = [ctypes.POINTER(ctypes.c_int64),
                                           ctypes.c_size_t]
    lib.axon_start_nrt_profile.restype = ctypes.c_int64
    lib.axon_stop_nrt_profile.argtypes = [ctypes.c_char_p]
    lib.axon_stop_nrt_profile.restype = ctypes.c_int64

    @contextlib.contextmanager
    def _hook(output_dir, device_ids):
        import jax
        jax.devices()
        if device_ids:
            ids = (ctypes.c_int64 * len(device_ids))(*device_ids)
            rc = lib.axon_start_nrt_profile(ids, len(device_ids))
        else:
            rc = lib.axon_start_nrt_profile(None, 0)
        if rc != 0:
            raise RuntimeError(f"axon_start_nrt_profile rc={rc}")
        try:
            yield
        finally:
            n = lib.axon_stop_nrt_profile(str(output_dir).encode())
            print(f"profile: {n} file(s) written to {output_dir}",
                  file=sys.stderr)

    mod.set_axon_ntff_profile_hook(_hook)


def _run(in_maps, trace=False, **kw):
    if trace:
        _install_profile_hook()
    nc = _get_nc()
    res = run_bass_kernel_spmd(nc, in_maps, core_ids=list(range(NC)),
                               trace=trace, **kw)
    outs = [np.asarray(r["out"], np.float32) for r in res.results]
    prob = np.zeros((PN, ACT), np.float32)
    gnn = np.zeros((PN, EMB), np.float32)
    for c in range(NC):
        prob[c * SH:(c + 1) * SH, :] = outs[c][0:ACT, :].T
        gnn[c * SH:(c + 1) * SH, :] = outs[c][ACT:ACT + EMB, :].T
    return (prob[:NR], gnn[:ND]), res


def kernel(**inputs):
    in_maps = _prep_inputs(**inputs)
    (prob, gnn), _ = _run(in_maps, trace=False)
    return prob, gnn


# revision 10
# speedup vs baseline: 1.2853x; 1.2853x over previous
"""Trainium2 distributed kernel for nn_ActorGNNMLP (3-layer hetero GraphConv + MLP).

Approach
--------
Each DGL GraphConv is linear:  gconv(x) = S @ (x @ W) + b  with
S = D_in^-1/2 A D_out^-1/2 a dense normalized adjacency built host-side from
the edge indices only (pure index/layout preprocessing; all feature compute
runs on device).  Work is sharded over 8 NeuronCores by destination rows
(region/driver padded 5000 -> 5120 = 8*640).  Hidden states are kept in a
transposed [feat, node] layout on-chip so no on-device transposes are needed;
full hidden states are exchanged between layers with AllGather collectives.

All large operands are packed host-side into partition-major "images"
([128, ntiles*cols]) so each DMA descriptor moves C*cols contiguous bytes.
S_dd.T / S_rd.T shards are cached in SBUF across all three layers (they are
each used 3x).  bf16 everywhere on the matmul path, f32 PSUM accumulation.
"""

import sys

sys.path.insert(0, "/opt/trn_rl_repo")

import numpy as np
import ml_dtypes

from concourse import bass, bacc, mybir, tile
from concourse.bass_utils import run_bass_kernel_spmd

BF16 = ml_dtypes.bfloat16
F32 = mybir.dt.float32
BF = mybir.dt.bfloat16

NC = 8
NR, ND, NO, NPOI = 5000, 5000, 50000, 20000
PN = 5120          # padded region/driver count
SH = PN // NC      # 640 dst rows per core
KO = 50048         # padded order count   (391 * 128)
KP = 20096         # padded poi count     (157 * 128)
FD = 5025
FDP = 5120         # padded driver feature dim
HID, EMB, MLPH, ACT = 128, 64, 256, 26
NT = PN // 128     # 40 src tiles (region/driver)
NTO = KO // 128    # 391
NTP = KP // 128    # 157
NB = SH // 128     # 5 128-blocks per shard
CS = 4             # k-tiles per streamed S DMA chunk
CD = 8             # k-tiles per driver-transform DMA chunk

_CACHE = {}


# --------------------------------------------------------------------------
# host-side graph preprocessing (indices only)
# --------------------------------------------------------------------------

def _build_ST(src, dst, n_src, n_dst, n_src_pad):
    """S.T = (D_in^-1/2 A D_out^-1/2).T as [n_src_pad, PN] float32."""
    AT = np.zeros((n_src_pad, PN), np.float32)
    np.add.at(AT, (src, dst), 1.0)
    dout = np.maximum(np.bincount(src, minlength=n_src), 1).astype(np.float32) ** -0.5
    din = np.maximum(np.bincount(dst, minlength=n_dst), 1).astype(np.float32) ** -0.5
    AT[:n_src] *= dout[:, None]
    AT[:, :n_dst] *= din[None, :]
    return AT


def _to_img(M):
    """[nt*128, c] -> partition-major image [128, nt*c] (same dtype)."""
    n, c = M.shape
    nt = n // 128
    return np.ascontiguousarray(
        M.reshape(nt, 128, c).transpose(1, 0, 2).reshape(128, nt * c))


def _feat_img(x, n_pad):
    xp = np.zeros((n_pad, x.shape[1]), np.float32)
    xp[:x.shape[0]] = x
    return _to_img(xp.astype(BF16))


def _prep_inputs(x_region, x_driver, x_order, x_poi,
                 r2r_src, r2r_dst, d2r_src, d2r_dst, d2d_src, d2d_dst,
                 r2d_src, r2d_dst, o2r_src, o2r_dst, p2r_src, p2r_dst, params):
    p = {k: np.asarray(v, np.float32) for k, v in params.items()}
    bf = lambda a: np.ascontiguousarray(np.asarray(a).astype(BF16))

    sh = {}
    for name, (s, d, ns, nd_, npd) in {
        "srr": (r2r_src, r2r_dst, NR, NR, PN),
        "sdr": (d2r_src, d2r_dst, ND, NR, PN),
        "sdd": (d2d_src, d2d_dst, ND, ND, PN),
        "srd": (r2d_src, r2d_dst, NR, ND, PN),
        "sor": (o2r_src, o2r_dst, NO, NR, KO),
        "spr": (p2r_src, p2r_dst, NPOI, NR, KP),
    }.items():
        ST = _build_ST(np.asarray(s), np.asarray(d), ns, nd_, npd).astype(BF16)
        # per-core shard of columns, then partition-major image
        sh[name] = [_to_img(np.ascontiguousarray(ST[:, c * SH:(c + 1) * SH]))
                    for c in range(NC)]
        del ST

    # x_driver.T padded [FDP, PN] bf16, sharded by driver, imaged
    xdT = np.zeros((FDP, PN), np.float32)
    xdT[:FD, :ND] = np.asarray(x_driver, np.float32).T
    xdT = xdT.astype(BF16)
    sh["xd"] = [_to_img(np.ascontiguousarray(xdT[:, c * SH:(c + 1) * SH]))
                for c in range(NC)]
    del xdT

    xrT = np.zeros((4, PN), np.float32)
    xrT[:, :NR] = np.asarray(x_region, np.float32).T

    w1d = np.concatenate([np.pad(p["W1_dr"], ((0, FDP - FD), (0, 0))),
                          np.pad(p["W1_dd"], ((0, FDP - FD), (0, 0)))], axis=1)

    rep = {
        "xrT": bf(xrT),
        "xo_img": _feat_img(np.asarray(x_order, np.float32), KO),
        "xp_img": _feat_img(np.asarray(x_poi, np.float32), KP),
        "w1d": _to_img(w1d.astype(BF16)),
        "w1r": bf(np.concatenate([p["W1_rr"], p["W1_rd"]], axis=1)),
        "w1o": bf(p["W1_or"]),
        "w1p": bf(p["W1_pr"]),
        "w2r": bf(np.concatenate([p["W2_rr"], p["W2_rd"]], axis=1)),
        "w2d": bf(np.concatenate([p["W2_dd"], p["W2_dr"]], axis=1)),
        "w3r": bf(p["W3_rd"]),
        "w3d": bf(p["W3_dd"]),
        "wf1": bf(p["Wf1"]),
        "wf2_img": bf(p["Wf2"].reshape(2, 128, MLPH).transpose(1, 0, 2)
                      .reshape(128, 2 * MLPH)),
        "wf3_img": bf(p["Wf3"].reshape(2, 128, ACT).transpose(1, 0, 2)
                      .reshape(128, 2 * ACT)),
        "b128": np.ascontiguousarray(np.stack([
            p["b1_rr"], p["b1_dr"], p["b1_or"], p["b1_pr"],
            p["b1_dd"], p["b1_rd"],
            p["b2_rr"], p["b2_dr"], p["b2_dd"], p["b2_rd"]])),
        "b64": np.ascontiguousarray(np.stack([p["b3_dd"], p["b3_rd"]])),
        "bf12": np.ascontiguousarray(np.stack([p["bf1"], p["bf2"]])),
        "bf3v": np.ascontiguousarray(p["bf3"][None, :]),
    }

    in_maps = []
    for c in range(NC):
        m = {k: v[c] for k, v in sh.items()}
        m.update(rep)
        in_maps.append(m)
    return in_maps


# --------------------------------------------------------------------------
# device kernel
# --------------------------------------------------------------------------

def _declare(nc):
    d = {}
    specs = {
        "srr": ([128, NT * SH], BF), "sdr": ([128, NT * SH], BF),
        "sdd": ([128, NT * SH], BF), "srd": ([128, NT * SH], BF),
        "sor": ([128, NTO * SH], BF), "spr": ([128, NTP * SH], BF),
        "xd": ([128, NT * SH], BF), "xrT": ([4, PN], BF),
        "xo_img": ([128, NTO * 6], BF), "xp_img": ([128, NTP * 4], BF),
        "w1d": ([128, NT * 256], BF), "w1r": ([4, 256], BF),
        "w1o": ([6, 128], BF), "w1p": ([4, 128], BF),
        "w2r": ([128, 256], BF), "w2d": ([128, 256], BF),
        "w3r": ([128, 64], BF), "w3d": ([128, 64], BF),
        "wf1": ([64, 256], BF), "wf2_img": ([128, 512], BF),
        "wf3_img": ([128, 2 * ACT], BF),
        "b128": ([10, 128], F32), "b64": ([2, 64], F32),
        "bf12": ([2, 256], F32), "bf3v": ([1, ACT], F32),
    }
    for k, (shape, dt) in specs.items():
        d[k] = nc.dram_tensor(k, shape, dt, kind="ExternalInput")
    d["out"] = nc.dram_tensor("out", [ACT + EMB, SH], F32, kind="ExternalOutput")
    return d


def _build(nc):
    d = _declare(nc)
    RG = [list(range(NC))]
    RELU = mybir.ActivationFunctionType.Relu

    with tile.TileContext(nc) as tc:
        with (
            tc.tile_pool(name="const", bufs=1) as cp,
            tc.tile_pool(name="dram", bufs=1, space="DRAM") as dp,
            tc.tile_pool(name="scache", bufs=1) as scp,
        ):
            # ---- constants -------------------------------------------------
            ones = cp.tile([26, 1], F32, name="ones")
            nc.vector.memset(ones[:], 1.0)

            bvecs = {}
            with tc.tile_pool(name="cpsum", bufs=2, space="PSUM") as cps:
                def bias_vec(name, dram_ap, nrows, rows):
                    src = cp.tile([nrows, rows], F32, name=name + "_src")
                    nc.sync.dma_start(src[:], dram_ap)
                    ps = cps.tile([rows, 1], F32, name="bps", tag="bps")
                    nc.tensor.matmul(ps[:], src[:], ones[:nrows, :],
                                     start=True, stop=True)
                    sb = cp.tile([rows, 1], F32, name=name)
                    nc.scalar.copy(sb[:], ps[:])
                    bvecs[name] = sb

                bias_vec("b_hr", d["b128"][0:4, :], 4, 128)
                bias_vec("b_hd", d["b128"][4:6, :], 2, 128)
                bias_vec("b_nr", d["b128"][6:8, :], 2, 128)
                bias_vec("b_nd", d["b128"][8:10, :], 2, 128)
                bias_vec("b_g", d["b64"][0:2, :], 2, 64)
                bias_vec("b_f1a", d["bf12"][0:1, 0:128], 1, 128)
                bias_vec("b_f1b", d["bf12"][0:1, 128:256], 1, 128)
                bias_vec("b_f2a", d["bf12"][1:2, 0:128], 1, 128)
                bias_vec("b_f2b", d["bf12"][1:2, 128:256], 1, 128)
                bias_vec("b_f3", d["bf3v"][0:1, :], 1, ACT)

            def loadw(name, shape):
                t = cp.tile(shape, BF, name=name + "_sb")
                nc.sync.dma_start(t[:], d[name][:])
                return t

            w1r_bf = loadw("w1r", [4, 256])
            w1o_bf = loadw("w1o", [6, 128])
            w1p_bf = loadw("w1p", [4, 128])
            w2r_bf = loadw("w2r", [128, 256])
            w2d_bf = loadw("w2d", [128, 256])
            w3r_bf = loadw("w3r", [128, 64])
            w3d_bf = loadw("w3d", [128, 64])
            wf1_bf = loadw("wf1", [64, 256])
            wf2_bf = loadw("wf2_img", [128, 512])
            wf3_bf = loadw("wf3_img", [128, 2 * ACT])
            xr_bf = loadw("xrT", [4, PN])
            xo_bf = loadw("xo_img", [128, NTO * 6])
            xp_bf = loadw("xp_img", [128, NTP * 4])

            # ---- S_dd / S_rd shards cached in SBUF (used by L1+L2+L3) -----
            sdd_full = scp.tile([128, NT * SH], BF, name="sdd_full")
            srd_full = scp.tile([128, NT * SH], BF, name="srd_full")
            for j in range(NT // CD):
                lo, hi = j * CD * SH, (j + 1) * CD * SH
                nc.sync.dma_start(sdd_full[:, lo:hi], d["sdd"][:, lo:hi])
                nc.sync.dma_start(srd_full[:, lo:hi], d["srd"][:, lo:hi])

            def sdd_t(k):
                return sdd_full[:, k * SH:(k + 1) * SH]

            def srd_t(k):
                return srd_full[:, k * SH:(k + 1) * SH]

            # ---- collective buffers ---------------------------------------
            cc_td_in = dp.tile([SH, 256], BF, name="cc_td_in")
            cc_td_out = dp.tile([PN, 256], BF, name="cc_td_out",
                                addr_space="Shared")
            cc_h1_in = dp.tile([256, SH], BF, name="cc_h1_in")
            cc_h1_out = dp.tile([256 * NC, SH], BF, name="cc_h1_out",
                                addr_space="Shared")
            cc_h2_in = dp.tile([256, SH], BF, name="cc_h2_in")
            cc_h2_out = dp.tile([256 * NC, SH], BF, name="cc_h2_out",
                                addr_space="Shared")

            # ---- phase A: driver transform + AG ---------------------------
            with (
                tc.tile_pool(name="xdp", bufs=3) as xdp,
                tc.tile_pool(name="tdps", bufs=1, space="PSUM") as tdpsp,
                tc.tile_pool(name="tdsb", bufs=1) as tdsbp,
            ):
                td_ps = [tdpsp.tile([128, 256], F32, name=f"td_ps{m}")
                         for m in range(NB)]
                for j in range(NT // CD):
                    xdc = xdp.tile([128, CD * SH], BF, name="xdc")
                    nc.sync.dma_start(
                        xdc[:], d["xd"][:, j * CD * SH:(j + 1) * CD * SH])
                    w1c = xdp.tile([128, CD * 256], BF, name="w1c")
                    nc.sync.dma_start(
                        w1c[:], d["w1d"][:, j * CD * 256:(j + 1) * CD * 256])
                    for kk in range(CD):
                        k = j * CD + kk
                        for m in range(NB):
                            nc.tensor.matmul(
                                td_ps[m][:],
                                xdc[:, kk * SH + m * 128: kk * SH + (m + 1) * 128],
                                w1c[:, kk * 256:(kk + 1) * 256],
                                start=(k == 0), stop=(k == NT - 1))
                for m in range(NB):
                    tds = tdsbp.tile([128, 256], BF, name=f"tds{m}")
                    nc.vector.tensor_copy(tds[:], td_ps[m][:])
                    nc.sync.dma_start(cc_td_in[m * 128:(m + 1) * 128, :], tds[:])
                nc.gpsimd.collective_compute(
                    "AllGather", mybir.AluOpType.bypass, replica_groups=RG,
                    ins=[cc_td_in[:].opt()], outs=[cc_td_out[:].opt()])

            # ---- phase B: layer-1 aggregation -----------------------------
            def agg_pair(ps_a, ps_b, lhsT, rhs, first):
                nc.tensor.matmul(ps_a[:], lhsT, rhs[:, 0:512],
                                 start=first, stop=False)
                nc.tensor.matmul(ps_b[:], lhsT, rhs[:, 512:SH],
                                 start=first, stop=False)

            with (
                tc.tile_pool(name="accps", bufs=1, space="PSUM") as accp,
                tc.tile_pool(name="sstream", bufs=8) as ssp,
                tc.tile_pool(name="hsb", bufs=1) as hsbp,
            ):
                hr_a = accp.tile([128, 512], F32, name="hr_a")
                hr_b = accp.tile([128, SH - 512], F32, name="hr_b")
                hd_a = accp.tile([128, 512], F32, name="hd_a")
                hd_b = accp.tile([128, SH - 512], F32, name="hd_b")

                # o2r / p2r first: overlaps the td AllGather
                with tc.tile_pool(name="zps", bufs=1, space="PSUM") as zpsp:
                    zo_a = zpsp.tile([6, 512], F32, name="zo_a")
                    zo_b = zpsp.tile([6, SH - 512], F32, name="zo_b")
                    for j in range(NTO // CS + 1):
                        k0, k1 = j * CS, min((j + 1) * CS, NTO)
                        if k0 >= k1:
                            break
                        sc = ssp.tile([128, CS * SH], BF, name="schunk")
                        nc.sync.dma_start(sc[:, 0:(k1 - k0) * SH],
                                          d["sor"][:, k0 * SH:k1 * SH])
                        for k in range(k0, k1):
                            o = (k - k0) * SH
                            nc.tensor.matmul(
                                zo_a[:], xo_bf[:, k * 6:(k + 1) * 6],
                                sc[:, o:o + 512],
                                start=(k == 0), stop=(k == NTO - 1))
                            nc.tensor.matmul(
                                zo_b[:], xo_bf[:, k * 6:(k + 1) * 6],
                                sc[:, o + 512:o + SH],
                                start=(k == 0), stop=(k == NTO - 1))
                    zo = hsbp.tile([6, SH], BF, name="zo")
                    nc.vector.tensor_copy(zo[:, 0:512], zo_a[:])
                    nc.vector.tensor_copy(zo[:, 512:SH], zo_b[:])
                    agg_pair(hr_a, hr_b, w1o_bf[:], zo, True)

                    zp_a = zpsp.tile([4, 512], F32, name="zp_a", tag="zo_a")
                    zp_b = zpsp.tile([4, SH - 512], F32, name="zp_b", tag="zo_b")
                    for j in range(NTP // CS + 1):
                        k0, k1 = j * CS, min((j + 1) * CS, NTP)
                        if k0 >= k1:
                            break
                        sc = ssp.tile([128, CS * SH], BF, name="schunk")
                        nc.sync.dma_start(sc[:, 0:(k1 - k0) * SH],
                                          d["spr"][:, k0 * SH:k1 * SH])
                        for k in range(k0, k1):
                            o = (k - k0) * SH
                            nc.tensor.matmul(
                                zp_a[:], xp_bf[:, k * 4:(k + 1) * 4],
                                sc[:, o:o + 512],
                                start=(k == 0), stop=(k == NTP - 1))
                            nc.tensor.matmul(
                                zp_b[:], xp_bf[:, k * 4:(k + 1) * 4],
                                sc[:, o + 512:o + SH],
                                start=(k == 0), stop=(k == NTP - 1))
                    zp = hsbp.tile([4, SH], BF, name="zp")
                    nc.vector.tensor_copy(zp[:, 0:512], zp_a[:])
                    nc.vector.tensor_copy(zp[:, 512:SH], zp_b[:])
                    agg_pair(hr_a, hr_b, w1p_bf[:], zp, False)

                # region: transform inline + rr/rd aggregation
                with (
                    tc.tile_pool(name="trps", bufs=2, space="PSUM") as trps,
                    tc.tile_pool(name="trsb", bufs=3) as trsbp,
                ):
                    for j in range(NT // CS):
                        sc_rr = ssp.tile([128, CS * SH], BF, name="schunk")
                        nc.sync.dma_start(
                            sc_rr[:], d["srr"][:, j * CS * SH:(j + 1) * CS * SH])
                        for kk in range(CS):
                            s = j * CS + kk
                            ps = trps.tile([128, 256], F32, name="trps_t")
                            nc.tensor.matmul(ps[:],
                                             xr_bf[0:4, s * 128:(s + 1) * 128],
                                             w1r_bf[:], start=True, stop=True)
                            t = trsbp.tile([128, 256], BF, name="t_r")
                            nc.vector.tensor_copy(t[:], ps[:])
                            o = kk * SH
                            agg_pair(hr_a, hr_b, t[:, 0:128],
                                     sc_rr[:, o:o + SH], False)
                            agg_pair(hd_a, hd_b, t[:, 128:256],
                                     srd_t(s), s == 0)

                    # driver-src relations: dr (streamed) + dd (cached)
                    for j in range(NT // CS):
                        sc_dr = ssp.tile([128, CS * SH], BF, name="schunk")
                        nc.sync.dma_start(
                            sc_dr[:], d["sdr"][:, j * CS * SH:(j + 1) * CS * SH])
                        for kk in range(CS):
                            s = j * CS + kk
                            tdf = trsbp.tile([128, 256], BF, name="tdf")
                            nc.sync.dma_start(
                                tdf[:], cc_td_out[s * 128:(s + 1) * 128, :])
                            o = kk * SH
                            agg_pair(hr_a, hr_b, tdf[:, 0:128],
                                     sc_dr[:, o:o + SH], False)
                            agg_pair(hd_a, hd_b, tdf[:, 128:256],
                                     sdd_t(s), False)

                hr_sb = hsbp.tile([128, SH], BF, name="hr_sb")
                nc.scalar.activation(hr_sb[:, 0:512], hr_a[:], RELU,
                                     bias=bvecs["b_hr"][:])
                nc.scalar.activation(hr_sb[:, 512:SH], hr_b[:], RELU,
                                     bias=bvecs["b_hr"][:])
                hd_sb = hsbp.tile([128, SH], BF, name="hd_sb")
                nc.scalar.activation(hd_sb[:, 0:512], hd_a[:], RELU,
                                     bias=bvecs["b_hd"][:])
                nc.scalar.activation(hd_sb[:, 512:SH], hd_b[:], RELU,
                                     bias=bvecs["b_hd"][:])
                nc.sync.dma_start(cc_h1_in[0:128, :], hr_sb[:])
                nc.sync.dma_start(cc_h1_in[128:256, :], hd_sb[:])
                nc.gpsimd.collective_compute(
                    "AllGather", mybir.AluOpType.bypass, replica_groups=RG,
                    ins=[cc_h1_in[:].opt()], outs=[cc_h1_out[:].opt()])

            # ---- phase C: layer 2 -----------------------------------------
            with (
                tc.tile_pool(name="hblk", bufs=1) as hbp,
                tc.tile_pool(name="t2sb", bufs=4) as t2sbp,
                tc.tile_pool(name="t2ps", bufs=4, space="PSUM") as t2psp,
                tc.tile_pool(name="sstream2", bufs=8) as ssp2,
                tc.tile_pool(name="acc2", bufs=1, space="PSUM") as accp2,
                tc.tile_pool(name="hsb2", bufs=1) as hsbp2,
            ):
                hrB, hdB = [], []
                for c in range(NC):
                    hb = hbp.tile([128, SH], BF, name=f"hrB{c}")
                    nc.sync.dma_start(hb[:], cc_h1_out[256 * c:256 * c + 128, :])
                    hrB.append(hb)
                    db = hbp.tile([128, SH], BF, name=f"hdB{c}")
                    nc.sync.dma_start(db[:],
                                      cc_h1_out[256 * c + 128:256 * c + 256, :])
                    hdB.append(db)

                nr_a = accp2.tile([128, 512], F32, name="nr_a")
                nr_b = accp2.tile([128, SH - 512], F32, name="nr_b")
                nd_a = accp2.tile([128, 512], F32, name="nd_a")
                nd_b = accp2.tile([128, SH - 512], F32, name="nd_b")

                for j in range(NT // CS):
                    sc_rr = ssp2.tile([128, CS * SH], BF, name="s2chunk")
                    nc.sync.dma_start(
                        sc_rr[:], d["srr"][:, j * CS * SH:(j + 1) * CS * SH])
                    sc_dr = ssp2.tile([128, CS * SH], BF, name="s2chunk")
                    nc.sync.dma_start(
                        sc_dr[:], d["sdr"][:, j * CS * SH:(j + 1) * CS * SH])
                    for kk in range(CS):
                        s = j * CS + kk
                        c, sub = divmod(s, NB)
                        ps = t2psp.tile([128, 256], F32, name="t2ps_t")
                        nc.tensor.matmul(ps[:],
                                         hrB[c][:, sub * 128:(sub + 1) * 128],
                                         w2r_bf[:], start=True, stop=True)
                        t2r = t2sbp.tile([128, 256], BF, name="t2r")
                        nc.vector.tensor_copy(t2r[:], ps[:])
                        ps2 = t2psp.tile([128, 256], F32, name="t2ps_t")
                        nc.tensor.matmul(ps2[:],
                                         hdB[c][:, sub * 128:(sub + 1) * 128],
                                         w2d_bf[:], start=True, stop=True)
                        t2d = t2sbp.tile([128, 256], BF, name="t2d")
                        nc.vector.tensor_copy(t2d[:], ps2[:])
                        o = kk * SH
                        agg_pair(nr_a, nr_b, t2r[:, 0:128],
                                 sc_rr[:, o:o + SH], s == 0)
                        agg_pair(nd_a, nd_b, t2r[:, 128:256], srd_t(s), s == 0)
                        agg_pair(nr_a, nr_b, t2d[:, 128:256],
                                 sc_dr[:, o:o + SH], False)
                        agg_pair(nd_a, nd_b, t2d[:, 0:128], sdd_t(s), False)

                h2r_sb = hsbp2.tile([128, SH], BF, name="h2r_sb")
                nc.scalar.activation(h2r_sb[:, 0:512], nr_a[:], RELU,
                                     bias=bvecs["b_nr"][:])
                nc.scalar.activation(h2r_sb[:, 512:SH], nr_b[:], RELU,
                                     bias=bvecs["b_nr"][:])
                h2d_sb = hsbp2.tile([128, SH], BF, name="h2d_sb")
                nc.scalar.activation(h2d_sb[:, 0:512], nd_a[:], RELU,
                                     bias=bvecs["b_nd"][:])
                nc.scalar.activation(h2d_sb[:, 512:SH], nd_b[:], RELU,
                                     bias=bvecs["b_nd"][:])
                nc.sync.dma_start(cc_h2_in[0:128, :], h2r_sb[:])
                nc.sync.dma_start(cc_h2_in[128:256, :], h2d_sb[:])
                nc.gpsimd.collective_compute(
                    "AllGather", mybir.AluOpType.bypass, replica_groups=RG,
                    ins=[cc_h2_in[:].opt()], outs=[cc_h2_out[:].opt()])

            # ---- phase D: layer 3 (driver dst only) + MLP -----------------
            with (
                tc.tile_pool(name="hblk3", bufs=1) as hbp3,
                tc.tile_pool(name="t3sb", bufs=4) as t3sbp,
                tc.tile_pool(name="t3ps", bufs=2, space="PSUM") as t3psp,
                tc.tile_pool(name="acc3", bufs=1, space="PSUM") as accp3,
                tc.tile_pool(name="mlp", bufs=1) as mlpp,
                tc.tile_pool(name="mlpps", bufs=1, space="PSUM") as mlppsp,
            ):
                hr2B, hd2B = [], []
                for c in range(NC):
                    hb = hbp3.tile([128, SH], BF, name=f"hr2B{c}")
                    nc.sync.dma_start(hb[:], cc_h2_out[256 * c:256 * c + 128, :])
                    hr2B.append(hb)
                    db = hbp3.tile([128, SH], BF, name=f"hd2B{c}")
                    nc.sync.dma_start(db[:],
                                      cc_h2_out[256 * c + 128:256 * c + 256, :])
                    hd2B.append(db)

                g_a = accp3.tile([64, 512], F32, name="g_a")
                g_b = accp3.tile([64, SH - 512], F32, name="g_b")
                for s in range(NT):
                    c, sub = divmod(s, NB)
                    ps = t3psp.tile([128, 64], F32, name="t3ps_t")
                    nc.tensor.matmul(ps[:], hd2B[c][:, sub * 128:(sub + 1) * 128],
                                     w3d_bf[:], start=True, stop=True)
                    t3d = t3sbp.tile([128, 64], BF, name="t3d")
                    nc.vector.tensor_copy(t3d[:], ps[:])
                    ps2 = t3psp.tile([128, 64], F32, name="t3ps_t")
                    nc.tensor.matmul(ps2[:], hr2B[c][:, sub * 128:(sub + 1) * 128],
                                     w3r_bf[:], start=True, stop=True)
                    t3r = t3sbp.tile([128, 64], BF, name="t3r")
                    nc.vector.tensor_copy(t3r[:], ps2[:])
                    agg_pair(g_a, g_b, t3d[:], sdd_t(s), s == 0)
                    agg_pair(g_a, g_b, t3r[:], srd_t(s), False)

                g_bf = mlpp.tile([64, SH], BF, name="g_bf")
                g_f32 = mlpp.tile([64, SH], F32, name="g_f32")
                for (lo, hi, ps) in ((0, 512, g_a), (512, SH, g_b)):
                    nc.vector.tensor_scalar_add(g_bf[:, lo:hi], ps[:],
                                                bvecs["b_g"][:])
                    nc.vector.tensor_scalar_add(g_f32[:, lo:hi], ps[:],
                                                bvecs["b_g"][:])
                nc.sync.dma_start(d["out"][ACT:ACT + EMB, :], g_f32[:])

                # MLP head (transposed layout: [hidden, node])
                x1 = []
                for h in range(2):
                    ps_a = mlppsp.tile([128, 512], F32, name="m1a", tag="mlp_a")
                    ps_b = mlppsp.tile([128, SH - 512], F32, name="m1b",
                                       tag="mlp_b")
                    nc.tensor.matmul(ps_a[:], wf1_bf[0:64, h * 128:(h + 1) * 128],
                                     g_bf[:, 0:512], start=True, stop=True)
                    nc.tensor.matmul(ps_b[:], wf1_bf[0:64, h * 128:(h + 1) * 128],
                                     g_bf[:, 512:SH], start=True, stop=True)
                    x = mlpp.tile([128, SH], BF, name=f"x1_{h}")
                    bv = bvecs["b_f1a" if h == 0 else "b_f1b"]
                    nc.scalar.activation(x[:, 0:512], ps_a[:], RELU, bias=bv[:])
                    nc.scalar.activation(x[:, 512:SH], ps_b[:], RELU, bias=bv[:])
                    x1.append(x)

                x2 = []
                for h in range(2):
                    ps_a = mlppsp.tile([128, 512], F32, name="m2a", tag="mlp_a")
                    ps_b = mlppsp.tile([128, SH - 512], F32, name="m2b",
                                       tag="mlp_b")
                    for k in range(2):
                        lhsT = wf2_bf[:, 256 * k + 128 * h: 256 * k + 128 * h + 128]
                        nc.tensor.matmul(ps_a[:], lhsT, x1[k][:, 0:512],
                                         start=(k == 0), stop=(k == 1))
                        nc.tensor.matmul(ps_b[:], lhsT, x1[k][:, 512:SH],
                                         start=(k == 0), stop=(k == 1))
                    x = mlpp.tile([128, SH], BF, name=f"x2_{h}")
                    bv = bvecs["b_f2a" if h == 0 else "b_f2b"]
                    nc.scalar.activation(x[:, 0:512], ps_a[:], RELU, bias=bv[:])
                    nc.scalar.activation(x[:, 512:SH], ps_b[:], RELU, bias=bv[:])
                    x2.append(x)

                lg_a = mlppsp.tile([ACT, 512], F32, name="lga", tag="mlp_a")
                lg_b = mlppsp.tile([ACT, SH - 512], F32, name="lgb", tag="mlp_b")
                for k in range(2):
                    lhsT = wf3_bf[:, ACT * k:ACT * (k + 1)]
                    nc.tensor.matmul(lg_a[:], lhsT, x2[k][:, 0:512],
                                     start=(k == 0), stop=(k == 1))
                    nc.tensor.matmul(lg_b[:], lhsT, x2[k][:, 512:SH],
                                     start=(k == 0), stop=(k == 1))

                e_sb = mlpp.tile([ACT, SH], F32, name="e_sb")
                nc.scalar.activation(e_sb[:, 0:512], lg_a[:],
                                     mybir.ActivationFunctionType.Exp,
                                     bias=bvecs["b_f3"][:])
                nc.scalar.activation(e_sb[:, 512:SH], lg_b[:],
                                     mybir.ActivationFunctionType.Exp,
                                     bias=bvecs["b_f3"][:])
                s_a = mlppsp.tile([1, 512], F32, name="sa", tag="mlp_a")
                s_b = mlppsp.tile([1, SH - 512], F32, name="sb", tag="mlp_b")
                nc.tensor.matmul(s_a[:], ones[:ACT, :], e_sb[:, 0:512],
                                 start=True, stop=True)
                nc.tensor.matmul(s_b[:], ones[:ACT, :], e_sb[:, 512:SH],
                                 start=True, stop=True)
                r_sb = mlpp.tile([1, SH], F32, name="r_sb")
                nc.vector.reciprocal(r_sb[:, 0:512], s_a[:])
                nc.vector.reciprocal(r_sb[:, 512:SH], s_b[:])
                r_bc = mlpp.tile([ACT, SH], F32, name="r_bc")
                nc.gpsimd.partition_broadcast(r_bc[:], r_sb[:])
                prob = mlpp.tile([ACT, SH], F32, name="prob")
                nc.vector.tensor_mul(prob[:], e_sb[:], r_bc[:])
                nc.sync.dma_start(d["out"][0:ACT, :], prob[:])

    nc.compile()
    return nc


def _get_nc():
    if "nc" not in _CACHE:
        nc = bacc.Bacc("TRN2", target_bir_lowering=False, debug=False,
                       num_devices=NC)
        _CACHE["nc"] = _build(nc)
    return _CACHE["nc"]


def _install_profile_hook():
    """Provide antenv.axon_hooks (missing in this image) so that
    run_bass_kernel_spmd(trace=True) can capture NTFF profiles via the
    axon PJRT .so; also stub out the artifact upload (no object store)."""
    import types, ctypes, contextlib

    import concourse.bass_utils as bu
    bu.upload_artifacts = lambda tmpdir: str(tmpdir)

    import antenv
    if "antenv.axon_hooks" in sys.modules:
        return
    mod = types.ModuleType("antenv.axon_hooks")
    hook_box = [None]
    mod.set_axon_ntff_profile_hook = lambda h: hook_box.__setitem__(0, h)
    mod.get_axon_ntff_profile_hook = lambda: hook_box[0]
    sys.modules["antenv.axon_hooks"] = mod
    antenv.axon_hooks = mod

    so_path = "/opt/axon/libaxon_pjrt.so"
    lib = ctypes.CDLL(so_path)
    if not hasattr(lib, "axon_start_nrt_profile"):
        return
    lib.axon_start_nrt_profile.argtypes = [ctypes.POINTER(ctypes.c_int64),
                                           ctypes.c_size_t]
    lib.axon_start_nrt_profile.restype = ctypes.c_int64
    lib.axon_stop_nrt_profile.argtypes = [ctypes.c_char_p]
    lib.axon_stop_nrt_profile.restype = ctypes.c_int64

    @contextlib.contextmanager
    def _hook(output_dir, device_ids):
        import jax
        jax.devices()
        if device_ids:
            ids = (ctypes.c_int64 * len(device_ids))(*device_ids)
            rc = lib.axon_start_nrt_profile(ids, len(device_ids))
        else:
            rc = lib.axon_start_nrt_profile(None, 0)
        if rc != 0:
            raise RuntimeError(f"axon_start_nrt_profile rc={rc}")
        try:
            yield
        finally:
            n = lib.axon_stop_nrt_profile(str(output_dir).encode())
            print(f"profile: {n} file(s) written to {output_dir}",
                  file=sys.stderr)

    mod.set_axon_ntff_profile_hook(_hook)


def _run(in_maps, trace=False, **kw):
    if trace:
        _install_profile_hook()
    nc = _get_nc()
    res = run_bass_kernel_spmd(nc, in_maps, core_ids=list(range(NC)),
                               trace=trace, **kw)
    outs = [np.asarray(r["out"], np.float32) for r in res.results]
    prob = np.zeros((PN, ACT), np.float32)
    gnn = np.zeros((PN, EMB), np.float32)
    for c in range(NC):
        prob[c * SH:(c + 1) * SH, :] = outs[c][0:ACT, :].T
        gnn[c * SH:(c + 1) * SH, :] = outs[c][ACT:ACT + EMB, :].T
    return (prob[:NR], gnn[:ND]), res


def kernel(**inputs):
    in_maps = _prep_inputs(**inputs)
    (prob, gnn), _ = _run(in_maps, trace=False)
    return prob, gnn
